# revision 8
# baseline (speedup 1.0000x reference)
"""GNN message-passing encoder on 8 Trainium2 NeuronCores — V2.

Computation:
    h      = l2norm(relu(x @ W + b))                    [N, 128]
    neigh1 = segment_mean(h[src], dst)                  [N, 128]
    neigh2 = segment_mean(neigh1[src], dst)             [N, 128]
    out    = (h, 0.7*neigh1 + 0.3*neigh2)

Distribution: nodes range-sharded across 8 cores; fp16 feature tables
AllGather'd in two halves (int16 gather-index limit).

V2 aggregation design ("identity matmul"): within each core, dst nodes are
permuted so that each 128-node block has near-uniform in-degree per table
half (A/B membership of srcs is globally balanced first).  Gather tiles are
laid out one-edge-per-dst-slot: tile m of block b holds edge m of every dst
at its own partition; missing edges gather a reserved zero row.  The
segment-sum is then a plain PSUM accumulation: matmul with a constant
identity lhsT.  The 1/deg mean scale (and the 0.7/0.3 hop combine) is
applied per-partition at flush time on the scalar engine.  No per-edge
one-hot construction on the DVE at all.

The node permutation lives on the host: x is fed permuted, h_out/mh_out
come back permuted and are un-permuted in numpy.  Both hops share one set
of gather index arrays.
"""

import sys

for _p in ("/opt/trn_rl_repo",):
    if _p not in sys.path:
        sys.path.insert(0, _p)

import numpy as np

# ---------------------------------------------------------------- constants
N_NODES = 50000
N_EDGES = 800000
D_IN = 256
D_OUT = 128
NCORES = 8
LAM = 0.7
P = 128

NC_NODES = N_NODES // NCORES            # 6250 nodes per core
HALF = NC_NODES // 2                    # 3125 rows per table half per core
TAB_ROWS = NCORES * HALF                # 25000 data rows per table
ZROW = TAB_ROWS                         # reserved all-zero row
TAB_ALLOC = TAB_ROWS + 8
NB = (NC_NODES + P - 1) // P            # 49 dst blocks per core
MLP_SBK = 7                             # MLP blocks per x superblock
MLP_NSB = (NB + MLP_SBK - 1) // MLP_SBK
AG_SPLIT_BLOCK = HALF // P              # block 24 completes rows [0, HALF)
SB_TILE_TARGET = 144                    # aggregation tiles per superblock

assert TAB_ALLOC < 32768


# ---------------------------------------------------------------- host prep
def _balance_membership(src, dst):
    """Choose table half (A=0/B=1) per node, exactly HALF per core half,
    so each dst's in-edges split ~evenly between halves."""
    memb = ((np.arange(N_NODES) % NC_NODES) >= HALF).astype(np.int8)
    deg = np.bincount(dst, minlength=N_NODES)
    for _ in range(12):
        kB = np.bincount(dst, weights=memb[src].astype(np.float64),
                         minlength=N_NODES)
        imb = 2.0 * kB - deg                      # kB - kA per dst
        g = np.bincount(src, weights=np.sign(imb)[dst], minlength=N_NODES)
        swapped = 0
        for c in range(NCORES):
            lo = c * NC_NODES
            gm = g[lo:lo + NC_NODES]
            m = memb[lo:lo + NC_NODES]
            aidx = np.nonzero(m == 0)[0]
            bidx = np.nonzero(m == 1)[0]
            a_order = aidx[np.argsort(gm[aidx])]
            b_order = bidx[np.argsort(-gm[bidx])]
            k = min(len(a_order), len(b_order), NC_NODES // 16)
            good = (gm[a_order[:k]] < 0) & (gm[b_order[:k]] > 0)
            n = int(good.sum())
            if n:
                memb[lo + a_order[:k][good]] = 1
                memb[lo + b_order[:k][good]] = 0
            swapped += n
        if swapped == 0:
            break
    return memb


def _build_layout(src, dst):
    """Returns (layout, metas, perms).

    layout: core-uniform program structure (superblocks, tile counts).
    metas: per-core input arrays (idx slabs, recip columns).
    perms: per-core node permutation (slot -> original local node).
    """
    deg = np.bincount(dst, minlength=N_NODES).astype(np.float32)
    recip = (1.0 / np.maximum(deg, 1.0)).astype(np.float32)

    memb = _balance_membership(src, dst)
    kB_all = np.bincount(dst, weights=memb[src].astype(np.float64),
                         minlength=N_NODES).astype(np.int64)
    kA_all = np.bincount(dst, minlength=N_NODES) - kB_all

    # per-core permutation: A-membership nodes occupy slots [0, HALF) sorted
    # by (kA, kB); B-membership nodes occupy [HALF, NC) likewise.
    perms = []
    pos_global = np.empty(N_NODES, np.int64)   # node -> slot within its core
    for c in range(NCORES):
        lo = c * NC_NODES
        kAc = kA_all[lo:lo + NC_NODES]
        kBc = kB_all[lo:lo + NC_NODES]
        mc = memb[lo:lo + NC_NODES]
        aidx = np.nonzero(mc == 0)[0]
        bidx = np.nonzero(mc == 1)[0]
        a_sorted = aidx[np.lexsort((kBc[aidx], kAc[aidx]))]
        b_sorted = bidx[np.lexsort((kBc[bidx], kAc[bidx]))]
        perm = np.concatenate([a_sorted, b_sorted])
        perms.append(perm)
        inv = np.empty(NC_NODES, np.int64)
        inv[perm] = np.arange(NC_NODES)
        pos_global[lo:lo + NC_NODES] = inv

    # table index of node u (as src): its owner core's half-slab + slot
    u_pos = pos_global                           # slot position per node
    u_core = np.arange(N_NODES) // NC_NODES
    u_grp = (u_pos >= HALF)                      # == memb by construction
    tabidx_all = (u_core * HALF + np.where(u_grp, u_pos - HALF, u_pos)
                  ).astype(np.int16)

    # per-core per-block tile counts (max edge count per slot, per grp)
    e_grp = memb[src]                            # edge's table half
    MA = np.zeros((NCORES, NB), np.int64)
    MB = np.zeros((NCORES, NB), np.int64)
    owner = dst // NC_NODES
    core_edges = []
    for c in range(NCORES):
        sel = np.nonzero(owner == c)[0]
        slot = pos_global[dst[sel]]              # dst slot 0..NC-1
        g = e_grp[sel]
        tix = tabidx_all[src[sel]]
        kAp = np.bincount(slot[g == 0], minlength=NB * P)
        kBp = np.bincount(slot[g == 1], minlength=NB * P)
        ra = kAp.reshape(NB, P)
        rb = kBp.reshape(NB, P)
        MA[c] = ra.max(axis=1)
        MB[c] = rb.max(axis=1)
        core_edges.append((slot, g, tix))

    MAu = MA.max(axis=0)
    MBu = MB.max(axis=0)
    for b in range(NB):
        if MAu[b] + MBu[b] == 0:
            MAu[b] = 1

    # balanced superblocks
    sbs = []
    cur, cur_ts = [], 0
    for b in range(NB):
        t = int(MAu[b] + MBu[b])
        if cur and cur_ts + t > SB_TILE_TARGET:
            sbs.append(cur)
            cur, cur_ts = [], 0
        cur.append(b)
        cur_ts += t
    if cur:
        sbs.append(cur)

    # slot/tile enumeration per superblock: A tiles block-major, then B
    sb_meta = []
    ofsA = ofsB = 0
    TA_tot = TB_tot = 0
    for blocks in sbs:
        TaS = int(sum(MAu[b] for b in blocks))
        TbS = int(sum(MBu[b] for b in blocks))
        tiles = {}
        sA = 0
        sB = TaS
        for b in blocks:
            tiles[b] = []
        for b in blocks:
            for m in range(int(MAu[b])):
                tiles[b].append(sA)
                sA += 1
        for b in blocks:
            for m in range(int(MBu[b])):
                tiles[b].append(sB)
                sB += 1
        sb_meta.append(dict(blocks=blocks, TaS=TaS, TbS=TbS,
                            ofsA=ofsA, ofsB=ofsB, tiles=tiles))
        ofsA += TaS * 8                          # 8 idx cols per tile
        ofsB += TbS * 8
        TA_tot += TaS
        TB_tot += TbS
    SIA, SIB = ofsA, ofsB

    # per-core idx slabs and recip metadata
    metas = []
    for c in range(NCORES):
        slot, g, tix = core_edges[c]
        # m-index of each edge within its (slot, grp) bucket
        idx_a = np.full((16, SIA), ZROW, np.int16)
        idx_b = np.full((16, SIB), ZROW, np.int16)
        for gv, idx_sl, Mu in ((0, idx_a, MAu), (1, idx_b, MBu)):
            m_sel = g == gv
            s_sel = slot[m_sel]
            t_sel = tix[m_sel]
            order = np.argsort(s_sel, kind="stable")
            s_sorted = s_sel[order]
            t_sorted = t_sel[order]
            # running index within each slot
            first = np.searchsorted(s_sorted, np.arange(NB * P))
            mrun = np.arange(len(s_sorted)) - first[s_sorted]
            # dense [slot, m] -> tabidx
            Mmax = int(Mu.max()) if len(Mu) else 0
            dense = np.full((NB * P, max(Mmax, 1)), ZROW, np.int16)
            dense[s_sorted, mrun] = t_sorted
            # emit per superblock, block-major tile order
            for smeta in sb_meta:
                ofs = smeta["ofsA"] if gv == 0 else smeta["ofsB"]
                q = 0
                for b in smeta["blocks"]:
                    mu = int(Mu[b])
                    if mu == 0:
                        continue
                    # [mu, 128] tile-major values
                    vals = dense[b * P:(b + 1) * P, :mu].T.reshape(-1)
                    k = np.arange(len(vals)) + q
                    idx_sl[(k % 16), ofs + k // 16] = vals
                    q += len(vals)
        perm = perms[c]
        rc = np.zeros(NB * P, np.float32)
        rc[:NC_NODES] = recip[c * NC_NODES + perm]   # recip per slot
        rcol = np.ascontiguousarray(rc.reshape(NB, P).T)
        rcol03 = ((1.0 - LAM) * rcol).astype(np.float32)
        metas.append(dict(idx_a=np.tile(idx_a, (8, 1)),
                          idx_b=np.tile(idx_b, (8, 1)),
                          recip=np.ascontiguousarray(rcol),
                          recip03=np.ascontiguousarray(rcol03)))

    layout = dict(sbs=sb_meta, MAu=MAu, MBu=MBu, SIA=SIA, SIB=SIB,
                  TA=TA_tot, TB=TB_tot)
    return layout, metas, perms


def _layout_key(layout):
    key = [layout["SIA"], layout["SIB"], layout["TA"], layout["TB"]]
    for sb in layout["sbs"]:
        key += [sb["TaS"], sb["TbS"], sb["ofsA"], sb["ofsB"]]
        key.append(tuple(sb["blocks"]))
        for b in sb["blocks"]:
            key.append(tuple(sb["tiles"][b]))
    return tuple(key)


# ---------------------------------------------------------------- device IR
_PROGRAM_CACHE = {}


def _build_program(layout):
    from contextlib import ExitStack

    import concourse.bacc as bacc
    from concourse import mybir
    from concourse.bass import _add_dep_helper
    from concourse.tile import TileContext

    f32 = mybir.dt.float32
    f16 = mybir.dt.float16
    i16 = mybir.dt.int16
    Alu = mybir.AluOpType
    Act = mybir.ActivationFunctionType

    sbs = layout["sbs"]
    SIA = layout["SIA"]
    SIB = layout["SIB"]

    nc = bacc.Bacc("TRN2", target_bir_lowering=False, debug=False,
                   num_devices=NCORES, num_swdge_queues=4)

    # I/O
    xt_d = nc.dram_tensor("xt", [MLP_NSB, 2, P, MLP_SBK * P], f32,
                          kind="ExternalInput")
    w_d = nc.dram_tensor("wmat", [2, P, D_OUT], f32, kind="ExternalInput")
    bias_d = nc.dram_tensor("bias", [1, D_OUT], f32, kind="ExternalInput")
    ones_d = nc.dram_tensor("ones1", [1, P], f32, kind="ExternalInput")
    ident_d = nc.dram_tensor("ident", [P, P], f16, kind="ExternalInput")
    recip_d = nc.dram_tensor("recip", [P, NB], f32, kind="ExternalInput")
    recip03_d = nc.dram_tensor("recip03", [P, NB], f32, kind="ExternalInput")
    idxa_d = nc.dram_tensor("idx_a", [P, SIA], i16, kind="ExternalInput")
    idxb_d = nc.dram_tensor("idx_b", [P, SIB], i16, kind="ExternalInput")

    h_out_d = nc.dram_tensor("h_out", [NC_NODES, D_OUT], f32,
                             kind="ExternalOutput")
    mh_out_d = nc.dram_tensor("mh_out", [NC_NODES, D_OUT], f32,
                              kind="ExternalOutput")

    # internal DRAM
    hshard_d = nc.dram_tensor("hshard16", [NC_NODES, D_OUT], f16)
    n1shard_d = nc.dram_tensor("n1shard16", [NC_NODES, D_OUT], f16)
    htab_a = nc.dram_tensor("htab_a", [TAB_ALLOC, D_OUT], f16,
                            addr_space="Shared")
    htab_b = nc.dram_tensor("htab_b", [TAB_ALLOC, D_OUT], f16,
                            addr_space="Shared")
    ntab_a = nc.dram_tensor("ntab_a", [TAB_ALLOC, D_OUT], f16,
                            addr_space="Shared")
    ntab_b = nc.dram_tensor("ntab_b", [TAB_ALLOC, D_OUT], f16,
                            addr_space="Shared")

    rg = [list(range(NCORES))]

    with TileContext(nc) as tc, ExitStack() as ctx:
        const = ctx.enter_context(tc.tile_pool(name="const", bufs=1))
        meta = ctx.enter_context(tc.tile_pool(name="meta", bufs=1))
        xtp = ctx.enter_context(tc.tile_pool(name="xtp", bufs=2))
        featp = ctx.enter_context(tc.tile_pool(name="featp", bufs=3))
        accp = ctx.enter_context(tc.tile_pool(name="accp", bufs=1))
        work = ctx.enter_context(tc.tile_pool(name="work", bufs=3))
        outp = ctx.enter_context(tc.tile_pool(name="outp", bufs=4))
        psmlp = ctx.enter_context(tc.tile_pool(name="psmlp", bufs=3,
                                               space="PSUM"))
        pshop = ctx.enter_context(tc.tile_pool(name="pshop", bufs=4,
                                               space="PSUM"))

        # ---- constants / metadata
        ident_sb = const.tile([P, P], f16, tag="ident")
        nc.sync.dma_start(ident_sb[:], ident_d[:, :])
        w_sb = [const.tile([P, D_OUT], f32, tag=f"w{t}", name=f"w_sb{t}")
                for t in range(2)]
        for t in range(2):
            nc.sync.dma_start(w_sb[t][:], w_d[t])
        ones_sb = const.tile([1, P], f32, tag="ones")
        nc.sync.dma_start(ones_sb[:], ones_d[:, :])
        bias_sb = const.tile([1, D_OUT], f32, tag="bias")
        nc.sync.dma_start(bias_sb[:], bias_d[:, :])
        recip_sb = meta.tile([P, NB], f32, tag="recip")
        nc.sync.dma_start(recip_sb[:], recip_d[:, :])
        recip03_sb = meta.tile([P, NB], f32, tag="recip03")
        nc.sync.dma_start(recip03_sb[:], recip03_d[:, :])
        idxa_sb = meta.tile([P, SIA], i16, tag="idxa")
        nc.sync.dma_start(idxa_sb[:], idxa_d[:, :])
        idxb_sb = meta.tile([P, SIB], i16, tag="idxb")
        nc.sync.dma_start(idxb_sb[:], idxb_d[:, :])

        # zero row of each gather table
        zrow_sb = const.tile([1, D_OUT], f16, tag="zrow")
        nc.vector.memset(zrow_sb[:], 0.0)
        zdeps = {}
        for tab, nm in ((htab_a, "za"), (htab_b, "zb"),
                        (ntab_a, "na"), (ntab_b, "nb")):
            zdeps[nm] = nc.sync.dma_start(tab[ZROW:ZROW + 1, :], zrow_sb[:])

        acc_sb = accp.tile([P, NB * D_OUT], f16, tag="acc")

        # ---- phase 1: MLP  h = l2norm(relu(x @ W + b))
        ag_insts = {}

        def emit_ag(name, src_ap, dst_ap):
            inst = nc.gpsimd.collective_compute(
                "AllGather", Alu.bypass, replica_groups=rg,
                ins=[src_ap], outs=[dst_ap],
            )
            ag_insts[name] = inst
            return inst

        for s in range(MLP_NSB):
            xts = xtp.tile([P, 2, MLP_SBK * P], f32, tag="xts")
            for t in range(2):
                nc.sync.dma_start(xts[:, t, :], xt_d[s, t])
            for bl in range(MLP_SBK):
                B = s * MLP_SBK + bl
                if B >= NB:
                    break
                ps = psmlp.tile([P, D_OUT], f32, tag="psmlp")
                for t in range(2):
                    nc.tensor.matmul(
                        ps[:], lhsT=xts[:, t, bl * P:(bl + 1) * P],
                        rhs=w_sb[t][:], start=(t == 0), stop=False,
                    )
                nc.tensor.matmul(ps[:], lhsT=ones_sb[:], rhs=bias_sb[:],
                                 start=False, stop=True)
                hb = work.tile([P, D_OUT], f32, tag="hb")
                nc.scalar.activation(hb[:], ps[:], Act.Relu)
                sq = work.tile([P, D_OUT], f32, tag="sq")
                ns = work.tile([P, 1], f32, tag="ns")
                nc.scalar.activation(sq[:], hb[:], Act.Square, accum_out=ns[:])
                nsc = work.tile([P, 1], f32, tag="nsc")
                nc.vector.tensor_scalar(out=nsc[:], in0=ns[:], scalar1=1e-24,
                                        scalar2=None, op0=Alu.max)
                sqr = work.tile([P, 1], f32, tag="sqr")
                nc.scalar.activation(sqr[:], nsc[:], Act.Sqrt)
                rn = work.tile([P, 1], f32, tag="rn")
                nc.vector.reciprocal(rn[:], sqr[:])
                hO = outp.tile([P, D_OUT], f32, tag="hO")
                nc.scalar.activation(hO[:], hb[:], Act.Copy, scale=rn[:])
                h16 = outp.tile([P, D_OUT], f16, tag="h16")
                nc.scalar.activation(h16[:], hb[:], Act.Copy, scale=rn[:])
                rows = min(P, NC_NODES - B * P)
                nc.sync.dma_start(h_out_d[B * P:B * P + rows, :], hO[:rows, :])
                nc.sync.dma_start(hshard_d[B * P:B * P + rows, :],
                                  h16[:rows, :])
                if B == AG_SPLIT_BLOCK:
                    emit_ag("h_a", hshard_d[0:HALF, :], htab_a[0:TAB_ROWS, :])
        emit_ag("h_b", hshard_d[HALF:NC_NODES, :], htab_b[0:TAB_ROWS, :])

        # ---- phases 2/3: aggregation hops
        qctr = [0]
        _size_regs = {}

        def _size_reg(n):
            if n not in _size_regs:
                _size_regs[n] = nc.gpsimd.to_reg(n)
            return _size_regs[n]

        def emit_gather(fb, slot0, ntiles, tab, idx_sb, col0, deps, why):
            # 4-way split across all SWDGE queues
            bounds = [round(ntiles * i / 4) for i in range(5)]
            for t0, t1 in zip(bounds[:-1], bounds[1:]):
                if t1 <= t0:
                    continue
                n = (t1 - t0) * P
                gi = nc.gpsimd.dma_gather(
                    fb[:, slot0 + t0:slot0 + t1, :], tab[:, :],
                    idx_sb[:, col0 + t0 * 8:col0 + t1 * 8],
                    n, _size_reg(n), D_OUT, single_packet=False,
                    queue_num=qctr[0] % 4,
                )
                qctr[0] += 1
                for dep in deps:
                    _add_dep_helper(gi.ins, dep.ins, True, why)

        def flush1(B, ps):
            n16 = outp.tile([P, D_OUT], f16, tag="n16")
            nc.scalar.activation(n16[:], ps[:], Act.Copy,
                                 scale=recip_sb[:, B:B + 1])
            nc.scalar.activation(acc_sb[:, B * D_OUT:(B + 1) * D_OUT], ps[:],
                                 Act.Copy, scale=LAM / (1.0 - LAM))
            rows = min(P, NC_NODES - B * P)
            nc.sync.dma_start(n1shard_d[B * P:B * P + rows, :], n16[:rows, :])
            if B == AG_SPLIT_BLOCK:
                emit_ag("n_a", n1shard_d[0:HALF, :], ntab_a[0:TAB_ROWS, :])

        def flush2(B, ps):
            mh = outp.tile([P, D_OUT], f32, tag="mh")
            nc.scalar.activation(mh[:], ps[:], Act.Copy,
                                 scale=recip03_sb[:, B:B + 1])
            rows = min(P, NC_NODES - B * P)
            nc.sync.dma_start(mh_out_d[B * P:B * P + rows, :], mh[:rows, :])

        # hop 1: psum groups must be stopped — emit with stop on last tile
        def emit_hop_stopaware(tab_a, tab_b, dep_a, dep_b, flush,
                               add_acc=False):
            for smeta in sbs:
                TaS, TbS = smeta["TaS"], smeta["TbS"]
                TS = TaS + TbS
                fb = featp.tile([P, TS, D_OUT], f16, tag="fb")
                if TaS > 0:
                    emit_gather(fb, 0, TaS, tab_a, idxa_sb, smeta["ofsA"],
                                dep_a, "gather A")
                if TbS > 0:
                    emit_gather(fb, TaS, TbS, tab_b, idxb_sb, smeta["ofsB"],
                                dep_b, "gather B")
                for b in smeta["blocks"]:
                    slots = smeta["tiles"][b]
                    ps = pshop.tile([P, D_OUT], f32, tag="pshop")
                    for i, slot in enumerate(slots):
                        last = (i == len(slots) - 1) and not add_acc
                        nc.tensor.matmul(
                            ps[:], lhsT=ident_sb[:], rhs=fb[:, slot, :],
                            start=(i == 0), stop=last,
                        )
                    if add_acc:
                        nc.tensor.matmul(
                            ps[:], lhsT=ident_sb[:],
                            rhs=acc_sb[:, b * D_OUT:(b + 1) * D_OUT],
                            start=False, stop=True,
                        )
                    flush(b, ps)

        emit_hop_stopaware(htab_a, htab_b,
                           [ag_insts["h_a"], zdeps["za"]],
                           [ag_insts["h_b"], zdeps["zb"]], flush1)
        emit_ag("n_b", n1shard_d[HALF:NC_NODES, :], ntab_b[0:TAB_ROWS, :])
        emit_hop_stopaware(ntab_a, ntab_b,
                           [ag_insts["n_a"], zdeps["na"]],
                           [ag_insts["n_b"], zdeps["nb"]], flush2,
                           add_acc=True)

    nc.compile()
    return nc


# ---------------------------------------------------------------- entry
def _build_in_maps(x, W, b, metas, perms):
    ident = np.eye(P, dtype=np.float16)
    wmat = np.stack([W[0:P, :], W[P:2 * P, :]]).astype(np.float32)
    bias = b.reshape(1, D_OUT).astype(np.float32)
    ones1 = np.ones((1, P), np.float32)

    in_maps = []
    for c in range(NCORES):
        xs = x[c * NC_NODES:(c + 1) * NC_NODES][perms[c]]
        xs_pad = np.zeros((MLP_NSB * MLP_SBK * P, D_IN), np.float32)
        xs_pad[:NC_NODES] = xs
        xt = np.zeros((MLP_NSB, 2, P, MLP_SBK * P), np.float32)
        for s in range(MLP_NSB):
            chunk = xs_pad[s * MLP_SBK * P:(s + 1) * MLP_SBK * P]
            ct = np.ascontiguousarray(chunk.T)
            xt[s, 0] = ct[0:P]
            xt[s, 1] = ct[P:2 * P]
        m = metas[c]
        in_maps.append(
            dict(
                xt=xt, wmat=wmat, bias=bias, ones1=ones1, ident=ident,
                recip=m["recip"], recip03=m["recip03"],
                idx_a=m["idx_a"], idx_b=m["idx_b"],
            )
        )
    return in_maps


def kernel(x, W, b, src, dst):
    x = np.asarray(x, np.float32)
    W = np.asarray(W, np.float32)
    b = np.asarray(b, np.float32)
    src = np.asarray(src, np.int32)
    dst = np.asarray(dst, np.int32)

    layout, metas, perms = _build_layout(src, dst)
    key = _layout_key(layout)
    if key not in _PROGRAM_CACHE:
        _PROGRAM_CACHE[key] = _build_program(layout)
    nc = _PROGRAM_CACHE[key]
    in_maps = _build_in_maps(x, W, b, metas, perms)

    from concourse.bass_utils import run_bass_kernel_spmd

    res = run_bass_kernel_spmd(nc, in_maps, list(range(NCORES)))
    h = np.empty((N_NODES, D_OUT), np.float32)
    mh = np.empty((N_NODES, D_OUT), np.float32)
    for c in range(NCORES):
        h[c * NC_NODES + perms[c]] = res.results[c]["h_out"]
        mh[c * NC_NODES + perms[c]] = res.results[c]["mh_out"]
    return (h, mh)


# revision 11
# speedup vs baseline: 9.4754x; 9.4754x over previous
"""GNN message-passing encoder on 8 Trainium2 NeuronCores — V2.

Computation:
    h      = l2norm(relu(x @ W + b))                    [N, 128]
    neigh1 = segment_mean(h[src], dst)                  [N, 128]
    neigh2 = segment_mean(neigh1[src], dst)             [N, 128]
    out    = (h, 0.7*neigh1 + 0.3*neigh2)

Distribution: nodes range-sharded across 8 cores; fp16 feature tables
AllGather'd in two halves (int16 gather-index limit).

V2 aggregation design ("identity matmul"): within each core, dst nodes are
permuted so that each 128-node block has near-uniform in-degree per table
half (A/B membership of srcs is globally balanced first).  Gather tiles are
laid out one-edge-per-dst-slot: tile m of block b holds edge m of every dst
at its own partition; missing edges gather a reserved zero row.  The
segment-sum is then a plain PSUM accumulation: matmul with a constant
identity lhsT.  The 1/deg mean scale (and the 0.7/0.3 hop combine) is
applied per-partition at flush time on the scalar engine.  No per-edge
one-hot construction on the DVE at all.

The node permutation lives on the host: x is fed permuted, h_out/mh_out
come back permuted and are un-permuted in numpy.  Both hops share one set
of gather index arrays.
"""

import sys

for _p in ("/opt/trn_rl_repo",):
    if _p not in sys.path:
        sys.path.insert(0, _p)

import numpy as np

# ---------------------------------------------------------------- constants
N_NODES = 50000
N_EDGES = 800000
D_IN = 256
D_OUT = 128
NCORES = 8
LAM = 0.7
P = 128

NC_NODES = N_NODES // NCORES            # 6250 nodes per core
HALF = NC_NODES // 2                    # 3125 rows per table half per core
TAB_ROWS = NCORES * HALF                # 25000 data rows per table
ZROW = TAB_ROWS                         # reserved all-zero row
TAB_ALLOC = TAB_ROWS + 8
NB = (NC_NODES + P - 1) // P            # 49 dst blocks per core
MLP_SBK = 7                             # MLP blocks per x superblock
MLP_NSB = (NB + MLP_SBK - 1) // MLP_SBK
AG_SPLIT_BLOCK = HALF // P              # block 24 completes rows [0, HALF)
SB_TILE_TARGET = 144                    # aggregation tiles per superblock
GATHER_SPLIT = 2                        # sub-calls per (superblock, grp)
SORT_EDGES_BY_SRC = True                # per-dst edge order: ascending tabidx

assert TAB_ALLOC < 32768


# ---------------------------------------------------------------- host prep
def _balance_membership(src, dst):
    """Choose table half (A=0/B=1) per node, exactly HALF per core half,
    so each dst's in-edges split ~evenly between halves."""
    memb = ((np.arange(N_NODES) % NC_NODES) >= HALF).astype(np.int8)
    deg = np.bincount(dst, minlength=N_NODES)
    for _ in range(12):
        kB = np.bincount(dst, weights=memb[src].astype(np.float64),
                         minlength=N_NODES)
        imb = 2.0 * kB - deg                      # kB - kA per dst
        g = np.bincount(src, weights=np.sign(imb)[dst], minlength=N_NODES)
        swapped = 0
        for c in range(NCORES):
            lo = c * NC_NODES
            gm = g[lo:lo + NC_NODES]
            m = memb[lo:lo + NC_NODES]
            aidx = np.nonzero(m == 0)[0]
            bidx = np.nonzero(m == 1)[0]
            a_order = aidx[np.argsort(gm[aidx])]
            b_order = bidx[np.argsort(-gm[bidx])]
            k = min(len(a_order), len(b_order), NC_NODES // 16)
            good = (gm[a_order[:k]] < 0) & (gm[b_order[:k]] > 0)
            n = int(good.sum())
            if n:
                memb[lo + a_order[:k][good]] = 1
                memb[lo + b_order[:k][good]] = 0
            swapped += n
        if swapped == 0:
            break
    return memb


def _build_layout(src, dst):
    """Returns (layout, metas, perms).

    layout: core-uniform program structure (superblocks, tile counts).
    metas: per-core input arrays (idx slabs, recip columns).
    perms: per-core node permutation (slot -> original local node).
    """
    deg = np.bincount(dst, minlength=N_NODES).astype(np.float32)
    recip = (1.0 / np.maximum(deg, 1.0)).astype(np.float32)

    memb = _balance_membership(src, dst)
    kB_all = np.bincount(dst, weights=memb[src].astype(np.float64),
                         minlength=N_NODES).astype(np.int64)
    kA_all = np.bincount(dst, minlength=N_NODES) - kB_all

    # per-core permutation: A-membership nodes occupy slots [0, HALF) sorted
    # by (kA, kB); B-membership nodes occupy [HALF, NC) likewise.
    perms = []
    pos_global = np.empty(N_NODES, np.int64)   # node -> slot within its core
    for c in range(NCORES):
        lo = c * NC_NODES
        kAc = kA_all[lo:lo + NC_NODES]
        kBc = kB_all[lo:lo + NC_NODES]
        mc = memb[lo:lo + NC_NODES]
        aidx = np.nonzero(mc == 0)[0]
        bidx = np.nonzero(mc == 1)[0]
        a_sorted = aidx[np.lexsort((kBc[aidx], kAc[aidx]))]
        b_sorted = bidx[np.lexsort((kBc[bidx], kAc[bidx]))]
        perm = np.concatenate([a_sorted, b_sorted])
        perms.append(perm)
        inv = np.empty(NC_NODES, np.int64)
        inv[perm] = np.arange(NC_NODES)
        pos_global[lo:lo + NC_NODES] = inv

    # table index of node u (as src): its owner core's half-slab + slot
    u_pos = pos_global                           # slot position per node
    u_core = np.arange(N_NODES) // NC_NODES
    u_grp = (u_pos >= HALF)                      # == memb by construction
    tabidx_all = (u_core * HALF + np.where(u_grp, u_pos - HALF, u_pos)
                  ).astype(np.int16)

    # per-core per-block tile counts (max edge count per slot, per grp)
    e_grp = memb[src]                            # edge's table half
    MA = np.zeros((NCORES, NB), np.int64)
    MB = np.zeros((NCORES, NB), np.int64)
    owner = dst // NC_NODES
    core_edges = []
    for c in range(NCORES):
        sel = np.nonzero(owner == c)[0]
        slot = pos_global[dst[sel]]              # dst slot 0..NC-1
        g = e_grp[sel]
        tix = tabidx_all[src[sel]]
        kAp = np.bincount(slot[g == 0], minlength=NB * P)
        kBp = np.bincount(slot[g == 1], minlength=NB * P)
        ra = kAp.reshape(NB, P)
        rb = kBp.reshape(NB, P)
        MA[c] = ra.max(axis=1)
        MB[c] = rb.max(axis=1)
        core_edges.append((slot, g, tix))

    MAu = MA.max(axis=0)
    MBu = MB.max(axis=0)
    for b in range(NB):
        if MAu[b] + MBu[b] == 0:
            MAu[b] = 1

    # balanced superblocks
    sbs = []
    cur, cur_ts = [], 0
    for b in range(NB):
        t = int(MAu[b] + MBu[b])
        if cur and cur_ts + t > SB_TILE_TARGET:
            sbs.append(cur)
            cur, cur_ts = [], 0
        cur.append(b)
        cur_ts += t
    if cur:
        sbs.append(cur)

    # slot/tile enumeration per superblock: A tiles block-major, then B
    sb_meta = []
    ofsA = ofsB = 0
    TA_tot = TB_tot = 0
    for blocks in sbs:
        TaS = int(sum(MAu[b] for b in blocks))
        TbS = int(sum(MBu[b] for b in blocks))
        tiles = {}
        sA = 0
        sB = TaS
        for b in blocks:
            tiles[b] = []
        for b in blocks:
            for m in range(int(MAu[b])):
                tiles[b].append(sA)
                sA += 1
        for b in blocks:
            for m in range(int(MBu[b])):
                tiles[b].append(sB)
                sB += 1
        sb_meta.append(dict(blocks=blocks, TaS=TaS, TbS=TbS,
                            ofsA=ofsA, ofsB=ofsB, tiles=tiles))
        ofsA += TaS * 8                          # 8 idx cols per tile
        ofsB += TbS * 8
        TA_tot += TaS
        TB_tot += TbS
    SIA, SIB = ofsA, ofsB

    # per-core idx slabs and recip metadata
    metas = []
    for c in range(NCORES):
        slot, g, tix = core_edges[c]
        # m-index of each edge within its (slot, grp) bucket
        idx_a = np.full((16, SIA), ZROW, np.int16)
        idx_b = np.full((16, SIB), ZROW, np.int16)
        for gv, idx_sl, Mu in ((0, idx_a, MAu), (1, idx_b, MBu)):
            m_sel = g == gv
            s_sel = slot[m_sel]
            t_sel = tix[m_sel]
            if SORT_EDGES_BY_SRC:
                order = np.lexsort((t_sel, s_sel))
            else:
                order = np.argsort(s_sel, kind="stable")
            s_sorted = s_sel[order]
            t_sorted = t_sel[order]
            # running index within each slot
            first = np.searchsorted(s_sorted, np.arange(NB * P))
            mrun = np.arange(len(s_sorted)) - first[s_sorted]
            # dense [slot, m] -> tabidx
            Mmax = int(Mu.max()) if len(Mu) else 0
            dense = np.full((NB * P, max(Mmax, 1)), ZROW, np.int16)
            dense[s_sorted, mrun] = t_sorted
            # emit per superblock, block-major tile order
            for smeta in sb_meta:
                ofs = smeta["ofsA"] if gv == 0 else smeta["ofsB"]
                q = 0
                for b in smeta["blocks"]:
                    mu = int(Mu[b])
                    if mu == 0:
                        continue
                    # [mu, 128] tile-major values
                    vals = dense[b * P:(b + 1) * P, :mu].T.reshape(-1)
                    k = np.arange(len(vals)) + q
                    idx_sl[(k % 16), ofs + k // 16] = vals
                    q += len(vals)
        perm = perms[c]
        rc = np.zeros(NB * P, np.float32)
        rc[:NC_NODES] = recip[c * NC_NODES + perm]   # recip per slot
        rcol = np.ascontiguousarray(rc.reshape(NB, P).T)
        rcol03 = ((1.0 - LAM) * rcol).astype(np.float32)
        metas.append(dict(idx_a=np.tile(idx_a, (8, 1)),
                          idx_b=np.tile(idx_b, (8, 1)),
                          recip=np.ascontiguousarray(rcol),
                          recip03=np.ascontiguousarray(rcol03)))

    layout = dict(sbs=sb_meta, MAu=MAu, MBu=MBu, SIA=SIA, SIB=SIB,
                  TA=TA_tot, TB=TB_tot)
    return layout, metas, perms


def _layout_key(layout):
    key = [layout["SIA"], layout["SIB"], layout["TA"], layout["TB"]]
    for sb in layout["sbs"]:
        key += [sb["TaS"], sb["TbS"], sb["ofsA"], sb["ofsB"]]
        key.append(tuple(sb["blocks"]))
        for b in sb["blocks"]:
            key.append(tuple(sb["tiles"][b]))
    return tuple(key)


# ---------------------------------------------------------------- device IR
_PROGRAM_CACHE = {}


def _build_program(layout):
    from contextlib import ExitStack

    import concourse.bacc as bacc
    from concourse import mybir
    from concourse.bass import _add_dep_helper
    from concourse.tile import TileContext

    f32 = mybir.dt.float32
    f16 = mybir.dt.float16
    i16 = mybir.dt.int16
    Alu = mybir.AluOpType
    Act = mybir.ActivationFunctionType

    sbs = layout["sbs"]
    SIA = layout["SIA"]
    SIB = layout["SIB"]

    nc = bacc.Bacc("TRN2", target_bir_lowering=False, debug=False,
                   num_devices=NCORES, num_swdge_queues=4)

    # I/O
    xt_d = nc.dram_tensor("xt", [MLP_NSB, 2, P, MLP_SBK * P], f32,
                          kind="ExternalInput")
    w_d = nc.dram_tensor("wmat", [2, P, D_OUT], f32, kind="ExternalInput")
    bias_d = nc.dram_tensor("bias", [1, D_OUT], f32, kind="ExternalInput")
    ones_d = nc.dram_tensor("ones1", [1, P], f32, kind="ExternalInput")
    ident_d = nc.dram_tensor("ident", [P, P], f16, kind="ExternalInput")
    recip_d = nc.dram_tensor("recip", [P, NB], f32, kind="ExternalInput")
    recip03_d = nc.dram_tensor("recip03", [P, NB], f32, kind="ExternalInput")
    idxa_d = nc.dram_tensor("idx_a", [P, SIA], i16, kind="ExternalInput")
    idxb_d = nc.dram_tensor("idx_b", [P, SIB], i16, kind="ExternalInput")

    h_out_d = nc.dram_tensor("h_out", [NC_NODES, D_OUT], f32,
                             kind="ExternalOutput")
    mh_out_d = nc.dram_tensor("mh_out", [NC_NODES, D_OUT], f32,
                              kind="ExternalOutput")

    # internal DRAM
    hshard_d = nc.dram_tensor("hshard16", [NC_NODES, D_OUT], f16)
    n1shard_d = nc.dram_tensor("n1shard16", [NC_NODES, D_OUT], f16)
    htab_a = nc.dram_tensor("htab_a", [TAB_ALLOC, D_OUT], f16,
                            addr_space="Shared")
    htab_b = nc.dram_tensor("htab_b", [TAB_ALLOC, D_OUT], f16,
                            addr_space="Shared")
    ntab_a = nc.dram_tensor("ntab_a", [TAB_ALLOC, D_OUT], f16,
                            addr_space="Shared")
    ntab_b = nc.dram_tensor("ntab_b", [TAB_ALLOC, D_OUT], f16,
                            addr_space="Shared")

    rg = [list(range(NCORES))]

    with TileContext(nc) as tc, ExitStack() as ctx:
        const = ctx.enter_context(tc.tile_pool(name="const", bufs=1))
        meta = ctx.enter_context(tc.tile_pool(name="meta", bufs=1))
        xtp = ctx.enter_context(tc.tile_pool(name="xtp", bufs=2))
        featp = ctx.enter_context(tc.tile_pool(name="featp", bufs=3))
        accp = ctx.enter_context(tc.tile_pool(name="accp", bufs=1))
        work = ctx.enter_context(tc.tile_pool(name="work", bufs=3))
        outp = ctx.enter_context(tc.tile_pool(name="outp", bufs=4))
        psmlp = ctx.enter_context(tc.tile_pool(name="psmlp", bufs=3,
                                               space="PSUM"))
        pshop = ctx.enter_context(tc.tile_pool(name="pshop", bufs=4,
                                               space="PSUM"))

        # ---- constants / metadata
        ident_sb = const.tile([P, P], f16, tag="ident")
        nc.sync.dma_start(ident_sb[:], ident_d[:, :])
        w_sb = [const.tile([P, D_OUT], f32, tag=f"w{t}", name=f"w_sb{t}")
                for t in range(2)]
        for t in range(2):
            nc.sync.dma_start(w_sb[t][:], w_d[t])
        ones_sb = const.tile([1, P], f32, tag="ones")
        nc.sync.dma_start(ones_sb[:], ones_d[:, :])
        bias_sb = const.tile([1, D_OUT], f32, tag="bias")
        nc.sync.dma_start(bias_sb[:], bias_d[:, :])
        recip_sb = meta.tile([P, NB], f32, tag="recip")
        nc.sync.dma_start(recip_sb[:], recip_d[:, :])
        recip03_sb = meta.tile([P, NB], f32, tag="recip03")
        nc.sync.dma_start(recip03_sb[:], recip03_d[:, :])
        idxa_sb = meta.tile([P, SIA], i16, tag="idxa")
        nc.sync.dma_start(idxa_sb[:], idxa_d[:, :])
        idxb_sb = meta.tile([P, SIB], i16, tag="idxb")
        nc.sync.dma_start(idxb_sb[:], idxb_d[:, :])

        # zero row of each gather table
        zrow_sb = const.tile([1, D_OUT], f16, tag="zrow")
        nc.vector.memset(zrow_sb[:], 0.0)
        zdeps = {}
        for tab, nm in ((htab_a, "za"), (htab_b, "zb"),
                        (ntab_a, "na"), (ntab_b, "nb")):
            zdeps[nm] = nc.sync.dma_start(tab[ZROW:ZROW + 1, :], zrow_sb[:])

        acc_sb = accp.tile([P, NB * D_OUT], f16, tag="acc")

        # ---- phase 1: MLP  h = l2norm(relu(x @ W + b))
        ag_insts = {}

        def emit_ag(name, src_ap, dst_ap):
            inst = nc.gpsimd.collective_compute(
                "AllGather", Alu.bypass, replica_groups=rg,
                ins=[src_ap], outs=[dst_ap],
            )
            ag_insts[name] = inst
            return inst

        for s in range(MLP_NSB):
            xts = xtp.tile([P, 2, MLP_SBK * P], f32, tag="xts")
            for t in range(2):
                nc.sync.dma_start(xts[:, t, :], xt_d[s, t])
            for bl in range(MLP_SBK):
                B = s * MLP_SBK + bl
                if B >= NB:
                    break
                ps = psmlp.tile([P, D_OUT], f32, tag="psmlp")
                for t in range(2):
                    nc.tensor.matmul(
                        ps[:], lhsT=xts[:, t, bl * P:(bl + 1) * P],
                        rhs=w_sb[t][:], start=(t == 0), stop=False,
                    )
                nc.tensor.matmul(ps[:], lhsT=ones_sb[:], rhs=bias_sb[:],
                                 start=False, stop=True)
                hb = work.tile([P, D_OUT], f32, tag="hb")
                nc.scalar.activation(hb[:], ps[:], Act.Relu)
                sq = work.tile([P, D_OUT], f32, tag="sq")
                ns = work.tile([P, 1], f32, tag="ns")
                nc.scalar.activation(sq[:], hb[:], Act.Square, accum_out=ns[:])
                nsc = work.tile([P, 1], f32, tag="nsc")
                nc.vector.tensor_scalar(out=nsc[:], in0=ns[:], scalar1=1e-24,
                                        scalar2=None, op0=Alu.max)
                sqr = work.tile([P, 1], f32, tag="sqr")
                nc.scalar.activation(sqr[:], nsc[:], Act.Sqrt)
                rn = work.tile([P, 1], f32, tag="rn")
                nc.vector.reciprocal(rn[:], sqr[:])
                hO = outp.tile([P, D_OUT], f32, tag="hO")
                nc.scalar.activation(hO[:], hb[:], Act.Copy, scale=rn[:])
                h16 = outp.tile([P, D_OUT], f16, tag="h16")
                nc.scalar.activation(h16[:], hb[:], Act.Copy, scale=rn[:])
                rows = min(P, NC_NODES - B * P)
                nc.sync.dma_start(h_out_d[B * P:B * P + rows, :], hO[:rows, :])
                nc.sync.dma_start(hshard_d[B * P:B * P + rows, :],
                                  h16[:rows, :])
                if B == AG_SPLIT_BLOCK:
                    emit_ag("h_a", hshard_d[0:HALF, :], htab_a[0:TAB_ROWS, :])
        emit_ag("h_b", hshard_d[HALF:NC_NODES, :], htab_b[0:TAB_ROWS, :])

        # ---- phases 2/3: aggregation hops
        qctr = [0]
        _size_regs = {}

        def _size_reg(n):
            if n not in _size_regs:
                _size_regs[n] = nc.gpsimd.to_reg(n)
            return _size_regs[n]

        def emit_gather(fb, slot0, ntiles, tab, idx_sb, col0, deps, why):
            # split across SWDGE queues
            nsplit = GATHER_SPLIT
            bounds = [round(ntiles * i / nsplit) for i in range(nsplit + 1)]
            for t0, t1 in zip(bounds[:-1], bounds[1:]):
                if t1 <= t0:
                    continue
                n = (t1 - t0) * P
                gi = nc.gpsimd.dma_gather(
                    fb[:, slot0 + t0:slot0 + t1, :], tab[:, :],
                    idx_sb[:, col0 + t0 * 8:col0 + t1 * 8],
                    n, _size_reg(n), D_OUT, single_packet=False,
                    queue_num=qctr[0] % 4,
                )
                qctr[0] += 1
                for dep in deps:
                    _add_dep_helper(gi.ins, dep.ins, True, why)

        def flush1(B, ps):
            n16 = outp.tile([P, D_OUT], f16, tag="n16")
            nc.scalar.activation(n16[:], ps[:], Act.Copy,
                                 scale=recip_sb[:, B:B + 1])
            nc.scalar.activation(acc_sb[:, B * D_OUT:(B + 1) * D_OUT], ps[:],
                                 Act.Copy, scale=LAM / (1.0 - LAM))
            rows = min(P, NC_NODES - B * P)
            nc.sync.dma_start(n1shard_d[B * P:B * P + rows, :], n16[:rows, :])
            if B == AG_SPLIT_BLOCK:
                emit_ag("n_a", n1shard_d[0:HALF, :], ntab_a[0:TAB_ROWS, :])

        def flush2(B, ps):
            mh = outp.tile([P, D_OUT], f32, tag="mh")
            nc.scalar.activation(mh[:], ps[:], Act.Copy,
                                 scale=recip03_sb[:, B:B + 1])
            rows = min(P, NC_NODES - B * P)
            nc.sync.dma_start(mh_out_d[B * P:B * P + rows, :], mh[:rows, :])

        # hop 1: psum groups must be stopped — emit with stop on last tile
        def emit_hop_stopaware(tab_a, tab_b, dep_a, dep_b, flush,
                               add_acc=False):
            for smeta in sbs:
                TaS, TbS = smeta["TaS"], smeta["TbS"]
                TS = TaS + TbS
                fb = featp.tile([P, TS, D_OUT], f16, tag="fb")
                if TaS > 0:
                    emit_gather(fb, 0, TaS, tab_a, idxa_sb, smeta["ofsA"],
                                dep_a, "gather A")
                if TbS > 0:
                    emit_gather(fb, TaS, TbS, tab_b, idxb_sb, smeta["ofsB"],
                                dep_b, "gather B")
                for b in smeta["blocks"]:
                    slots = smeta["tiles"][b]
                    ps = pshop.tile([P, D_OUT], f32, tag="pshop")
                    for i, slot in enumerate(slots):
                        last = (i == len(slots) - 1) and not add_acc
                        nc.tensor.matmul(
                            ps[:], lhsT=ident_sb[:], rhs=fb[:, slot, :],
                            start=(i == 0), stop=last,
                        )
                    if add_acc:
                        nc.tensor.matmul(
                            ps[:], lhsT=ident_sb[:],
                            rhs=acc_sb[:, b * D_OUT:(b + 1) * D_OUT],
                            start=False, stop=True,
                        )
                    flush(b, ps)

        emit_hop_stopaware(htab_a, htab_b,
                           [ag_insts["h_a"], zdeps["za"]],
                           [ag_insts["h_b"], zdeps["zb"]], flush1)
        emit_ag("n_b", n1shard_d[HALF:NC_NODES, :], ntab_b[0:TAB_ROWS, :])
        emit_hop_stopaware(ntab_a, ntab_b,
                           [ag_insts["n_a"], zdeps["na"]],
                           [ag_insts["n_b"], zdeps["nb"]], flush2,
                           add_acc=True)

    nc.compile()
    return nc


# ---------------------------------------------------------------- entry
def _build_in_maps(x, W, b, metas, perms):
    ident = np.eye(P, dtype=np.float16)
    wmat = np.stack([W[0:P, :], W[P:2 * P, :]]).astype(np.float32)
    bias = b.reshape(1, D_OUT).astype(np.float32)
    ones1 = np.ones((1, P), np.float32)

    in_maps = []
    for c in range(NCORES):
        xs = x[c * NC_NODES:(c + 1) * NC_NODES][perms[c]]
        xs_pad = np.zeros((MLP_NSB * MLP_SBK * P, D_IN), np.float32)
        xs_pad[:NC_NODES] = xs
        xt = np.zeros((MLP_NSB, 2, P, MLP_SBK * P), np.float32)
        for s in range(MLP_NSB):
            chunk = xs_pad[s * MLP_SBK * P:(s + 1) * MLP_SBK * P]
            ct = np.ascontiguousarray(chunk.T)
            xt[s, 0] = ct[0:P]
            xt[s, 1] = ct[P:2 * P]
        m = metas[c]
        in_maps.append(
            dict(
                xt=xt, wmat=wmat, bias=bias, ones1=ones1, ident=ident,
                recip=m["recip"], recip03=m["recip03"],
                idx_a=m["idx_a"], idx_b=m["idx_b"],
            )
        )
    return in_maps


def kernel(x, W, b, src, dst):
    x = np.asarray(x, np.float32)
    W = np.asarray(W, np.float32)
    b = np.asarray(b, np.float32)
    src = np.asarray(src, np.int32)
    dst = np.asarray(dst, np.int32)

    layout, metas, perms = _build_layout(src, dst)
    key = _layout_key(layout)
    if key not in _PROGRAM_CACHE:
        _PROGRAM_CACHE[key] = _build_program(layout)
    nc = _PROGRAM_CACHE[key]
    in_maps = _build_in_maps(x, W, b, metas, perms)

    from concourse.bass_utils import run_bass_kernel_spmd

    res = run_bass_kernel_spmd(nc, in_maps, list(range(NCORES)))
    h = np.empty((N_NODES, D_OUT), np.float32)
    mh = np.empty((N_NODES, D_OUT), np.float32)
    for c in range(NCORES):
        h[c * NC_NODES + perms[c]] = res.results[c]["h_out"]
        mh[c * NC_NODES + perms[c]] = res.results[c]["mh_out"]
    return (h, mh)
